# revision 7
# baseline (speedup 1.0000x reference)
"""Dense CRF loss kernel for Trainium2, 8 NeuronCores.

Problem: nn_CRFLoss — mean-field inference over two dense pairwise kernels
(Gaussian sigma=64, bilateral sigma=3/255) on a 96x96x21 image, 5 iterations,
plus a cross-entropy scalar broadcast into the output.

Strategy (sharding over the N=9216 pixel dimension, 1152 pixels per core):
 - Bilateral kernel Kb has 3-pixel spatial sigma -> banded: each core only
   materializes the [4224 x 1152] column strip (16-row margin) once, in bf16,
   resident in SBUF, generated on the TensorEngine (feature inner products)
   + ScalarEngine exp.
 - Gaussian kernel Kg = G (x) G is separable: never materialized. Kg @ Q is
   two small 96x96 matmuls per iteration (y-conv then x-conv) with a DRAM
   bounce to re-partition between them.
 - Each iteration: AllGather the [1152, 21] per-core Q strips -> full Q,
   banded Kb matmul accumulates msg^T in PSUM, Kg path adds its part,
   PE-transposes bring msg into the strip domain, fused softmax update.
 - The "-I" diagonal of both kernels is folded into the update as -2*Q.
 - softmax(-U - pair) == softmax(logits + 10*(msg - 2Q)) exactly (per-row
   constants cancel), so U is never materialized.
 - CE = mean(lse - logits[label]) via one-hot dot, partition-reduced by a
   ones-matmul, AllReduced across cores once, broadcast back via matmul.

Layouts:
 - strip domain: [96 partitions = x, free = (y_local 12, c 21)]
 - j domain (band/global): [128 partitions, tiles of 128 pixels]
 - Kb resident: [128, 33*1152] bf16, tile t columns = strip pixels (global
   pixel order), rows = band pixel j = r*1152 - 1536 + t*128 + p.
"""

import numpy as np
import ml_dtypes

import concourse.bass as bass
import concourse.bacc as bacc
import concourse.mybir as mybir
from concourse import tile
from concourse.bass_utils import run_bass_kernel_spmd

FP32 = mybir.dt.float32
BF16 = mybir.dt.bfloat16
AF = mybir.ActivationFunctionType
ALU = mybir.AluOpType
AX = mybir.AxisListType

H = W = 96
C = 21
N = H * W                 # 9216
NCORES = 8
STRIP = N // NCORES       # 1152
YL = H // NCORES // 1     # strip y-rows = 12
assert STRIP == YL * W
TS = STRIP // 128         # 9 tiles of 128 per strip
MB = 16                   # band margin in image rows
PAD = MB * W              # 1536
BAND = STRIP + 2 * PAD    # 4224
BT = BAND // 128          # 33 band tiles
QPAD_ROWS = N + 2 * PAD   # 12288
COMPAT = 10.0
N_ITERS = 5
FREE = YL * C             # 252 strip free size

# PSUM-bank-aligned free chunks of 1152 (fp32, 512 per 2KB bank)
CHUNKS3 = [(0, 512), (512, 512), (1024, 128)]
# chunks of 2016 within a [*, 2048] psum tile
CHUNKS4 = [(0, 512), (512, 512), (1024, 512), (1536, 480)]

_compiled = None


def build_nc():
    nc = bacc.Bacc("TRN2", target_bir_lowering=False, num_devices=NCORES)

    # per-core external inputs
    logits_d = nc.dram_tensor("logits_dev", [96, FREE], FP32, kind="ExternalInput")
    onehot_d = nc.dram_tensor("onehot_dev", [96, FREE], FP32, kind="ExternalInput")
    ft_d = nc.dram_tensor("ft_dev", [6, BAND], FP32, kind="ExternalInput")
    rt_d = nc.dram_tensor("rt_dev", [6, STRIP], FP32, kind="ExternalInput")
    biasb_d = nc.dram_tensor("biasb_dev", [128, BT], FP32, kind="ExternalInput")
    g_d = nc.dram_tensor("g_dev", [96, 96], FP32, kind="ExternalInput")
    ident_d = nc.dram_tensor("ident_dev", [32, 32], FP32, kind="ExternalInput")
    info_d = nc.dram_tensor("info_dev", [1, 2], mybir.dt.int32, kind="ExternalInput")
    out_d = nc.dram_tensor("out_strip", [96, FREE], FP32, kind="ExternalOutput")

    with tile.TileContext(nc) as tc:
        with (
            tc.tile_pool(name="sb", bufs=1) as sb,
            tc.tile_pool(name="dram", bufs=1, space="DRAM") as dram,
        ):
            # ---------------- SBUF persistent tiles ----------------
            logits_sb = sb.tile([96, FREE], FP32)
            onehot_sb = sb.tile([96, FREE], FP32)
            ft_sb = sb.tile([6, BAND], FP32)
            rt_sb = sb.tile([6, STRIP], FP32)
            biasb_sb = sb.tile([128, BT], FP32)
            g_sb = sb.tile([96, 96], FP32)
            ident_sb = sb.tile([32, 32], FP32)
            info_sb = sb.tile([1, 2], mybir.dt.int32)
            kb_sb = sb.tile([128, BT * STRIP], BF16)       # 76KB/part resident
            zero_sb = sb.tile([128, FREE], FP32)

            nc.sync.dma_start(logits_sb[:], logits_d[:])
            nc.sync.dma_start(onehot_sb[:], onehot_d[:])
            nc.sync.dma_start(ft_sb[:], ft_d[:])
            nc.sync.dma_start(rt_sb[:], rt_d[:])
            nc.sync.dma_start(biasb_sb[:], biasb_d[:])
            nc.sync.dma_start(g_sb[:], g_d[:])
            nc.sync.dma_start(ident_sb[:], ident_d[:])
            nc.sync.dma_start(info_sb[:], info_d[:])
            nc.vector.memset(zero_sb[:], 0.0)

            band0 = nc.values_load(
                info_sb[0:1, 0:1], min_val=0, max_val=(NCORES - 1) * STRIP,
                skip_runtime_bounds_check=True,
            )
            y0off = nc.values_load(
                info_sb[0:1, 1:2], min_val=0, max_val=(NCORES - 1) * FREE,
                skip_runtime_bounds_check=True,
            )

            # ---------------- DRAM scratch ----------------
            qout = dram.tile([STRIP, C], FP32)
            qall = dram.tile([N, C], FP32)
            qpad = dram.tile([QPAD_ROWS, C], FP32)
            t_dram = dram.tile([96, 96 * C], FP32)
            ce_in = dram.tile([1, 1], FP32)
            ce_out = dram.tile([1, 1], FP32)

            # zero the qpad margins (once; iterations only overwrite the middle)
            for r0 in (0, PAD + N):
                nc.sync.dma_start(
                    qpad[r0:r0 + PAD, :].rearrange("(t p) c -> p t c", p=128),
                    zero_sb[:, 0:(PAD // 128) * C].rearrange(
                        "p (t c) -> p t c", c=C),
                )

            # ---------------- Kb generation ----------------
            with tc.tile_pool(name="gen_ps", bufs=2, space="PSUM") as gen_ps:
                for t in range(BT):
                    ps_g = gen_ps.tile([128, STRIP], FP32, tag="gen")
                    for (o, w) in CHUNKS3:
                        nc.tensor.matmul(
                            ps_g[:, o:o + w],
                            ft_sb[0:6, t * 128:(t + 1) * 128],
                            rt_sb[0:6, o:o + w],
                            start=True, stop=True,
                        )
                    nc.scalar.activation(
                        kb_sb[:, t * STRIP:(t + 1) * STRIP], ps_g[:],
                        AF.Exp, bias=biasb_sb[:, t:t + 1],
                    )

            # ---------------- Q0 + CE ----------------
            e0 = sb.tile([96, FREE], FP32)
            s12 = sb.tile([96, YL], FP32)
            r12 = sb.tile([96, YL], FP32)
            lse = sb.tile([96, YL], FP32)
            dot = sb.tile([96, FREE], FP32)
            d12 = sb.tile([96, YL], FP32)
            ce96 = sb.tile([96, 1], FP32)
            ones96 = sb.tile([96, 1], FP32)
            ones1 = sb.tile([1, 96], FP32)
            ce_sb = sb.tile([1, 1], FP32)
            ce_all = sb.tile([1, 1], FP32)
            ce_bcast = sb.tile([96, 1], FP32)
            qA = sb.tile([96, FREE], FP32)
            qB = sb.tile([96, FREE], FP32)

            nc.vector.memset(ones96[:], 1.0)
            nc.vector.memset(ones1[:], 1.0)

            nc.scalar.activation(e0[:], logits_sb[:], AF.Exp)
            nc.vector.tensor_reduce(
                s12[:], e0[:].rearrange("p (y c) -> p y c", c=C),
                axis=AX.X, op=ALU.add,
            )
            nc.vector.reciprocal(r12[:], s12[:])
            for y in range(YL):
                nc.vector.tensor_scalar_mul(
                    qA[:, y * C:(y + 1) * C], e0[:, y * C:(y + 1) * C],
                    r12[:, y:y + 1],
                )
            # ce partial
            nc.scalar.activation(lse[:], s12[:], AF.Ln)
            nc.vector.tensor_mul(dot[:], logits_sb[:], onehot_sb[:])
            nc.vector.tensor_reduce(
                d12[:], dot[:].rearrange("p (y c) -> p y c", c=C),
                axis=AX.X, op=ALU.add,
            )
            nc.vector.tensor_sub(d12[:], lse[:], d12[:])
            nc.vector.tensor_reduce(ce96[:], d12[:], axis=AX.X, op=ALU.add)
            with tc.tile_pool(name="ce_ps", bufs=1, space="PSUM") as ce_ps:
                cep = ce_ps.tile([1, 1], FP32)
                nc.tensor.matmul(cep[:], ce96[:], ones96[:], start=True, stop=True)
                nc.scalar.activation(ce_sb[:], cep[:], AF.Copy, scale=1.0 / N)
            nc.sync.dma_start(ce_in[:], ce_sb[:])
            nc.gpsimd.collective_compute(
                "AllReduce", ALU.add,
                replica_groups=[list(range(NCORES))],
                ins=[ce_in.opt()], outs=[ce_out.opt()],
            )
            nc.sync.dma_start(ce_all[:], ce_out[:])
            with tc.tile_pool(name="ceb_ps", bufs=1, space="PSUM") as ceb_ps:
                cebp = ceb_ps.tile([96, 1], FP32)
                nc.tensor.matmul(cebp[:], ones1[:], ce_all[:], start=True, stop=True)
                nc.vector.tensor_copy(ce_bcast[:], cebp[:])

            # ---------------- iteration tiles ----------------
            qb32 = sb.tile([128, BT * C], FP32)
            qb16 = sb.tile([128, BT * C], BF16)
            qg = sb.tile([96, 96 * C], FP32)
            tcp = sb.tile([96, 96 * C], FP32)
            tp = sb.tile([96, 96 * C], FP32)
            msgT = sb.tile([21, STRIP], FP32)
            mg = sb.tile([96, FREE], FP32)
            msum = sb.tile([96, FREE], FP32)
            z1 = sb.tile([96, FREE], FP32)
            z2 = sb.tile([96, FREE], FP32)
            ez = sb.tile([96, FREE], FP32)
            negm = sb.tile([96, YL], FP32)

            q_cur, q_nxt = qA, qB

            with (
                tc.tile_pool(name="mm_ps", bufs=1, space="PSUM") as mm_ps,
                tc.tile_pool(name="kg_ps", bufs=1, space="PSUM") as kg_ps,
                tc.tile_pool(name="tr_ps", bufs=1, space="PSUM") as tr_ps,
            ):
                for it in range(N_ITERS):
                    # publish strip -> global order [1152, 21]
                    nc.sync.dma_start(
                        qout[:].rearrange("(y x) c -> x y c", x=96),
                        q_cur[:].rearrange("p (y c) -> p y c", c=C),
                    )
                    nc.gpsimd.collective_compute(
                        "AllGather", ALU.bypass,
                        replica_groups=[list(range(NCORES))],
                        ins=[qout.opt()], outs=[qall.opt()],
                    )
                    nc.sync.dma_start(qpad[PAD:PAD + N, :], qall[:])

                    # band read (dynamic offset) + cast
                    nc.gpsimd.dma_start(
                        qb32[:].rearrange("p (t c) -> p t c", c=C),
                        qpad[bass.ds(band0, BAND), :].rearrange(
                            "(t p) c -> p t c", p=128),
                    )
                    nc.vector.tensor_copy(qb16[:], qb32[:])

                    # ---- Kb matmul: psumT[c, s] += Q_band_t^T @ Kb_t
                    psT = mm_ps.tile([21, STRIP], FP32, tag="mm")
                    for t in range(BT):
                        for (o, w) in CHUNKS3:
                            nc.tensor.matmul(
                                psT[:, o:o + w],
                                qb16[:, t * C:(t + 1) * C],
                                kb_sb[:, t * STRIP + o: t * STRIP + o + w],
                                start=(t == 0), stop=(t == BT - 1),
                            )
                    nc.vector.tensor_copy(msgT[:], psT[:])

                    # ---- Kg path
                    nc.sync.dma_start(
                        qg[:].rearrange("p (x c) -> p x c", c=C),
                        qall[:].rearrange("(y x) c -> y x c", x=96))
                    psA = kg_ps.tile([96, 2048], FP32, tag="kg")
                    for (o, w) in CHUNKS4:
                        nc.tensor.matmul(
                            psA[:, o:o + w], g_sb[:], qg[:, o:o + w],
                            start=True, stop=True,
                        )
                    nc.scalar.activation(tcp[:], psA[:, 0:96 * C], AF.Copy)
                    nc.sync.dma_start(t_dram[:], tcp[:])
                    nc.sync.dma_start(
                        tp[:].rearrange("p (y c) -> p y c", c=C),
                        t_dram[:].rearrange("y (x c) -> x y c", c=C))
                    psB = kg_ps.tile([96, 2048], FP32, tag="kg")
                    for (o, w) in CHUNKS4:
                        nc.tensor.matmul(
                            psB[:, o:o + w], g_sb[:], tp[:, o:o + w],
                            start=True, stop=True,
                        )
                    # extract my strip (dynamic free offset) to SBUF
                    nc.scalar.activation(
                        mg[:], psB[:, bass.ds(y0off, FREE)], AF.Copy)

                    # ---- transpose msgT -> strip domain [96, (y, c)]
                    pstr = tr_ps.tile([96, FREE], FP32, tag="tr")
                    for y in range(YL):
                        nc.tensor.transpose(
                            pstr[:, y * C:(y + 1) * C],
                            msgT[:, y * 96:(y + 1) * 96],
                            ident_sb[0:21, 0:21],
                        )

                    # ---- combine + softmax update
                    nc.vector.tensor_add(msum[:], pstr[:], mg[:])
                    nc.vector.tensor_scalar_mul(z1[:], msum[:], COMPAT)
                    nc.vector.tensor_scalar_mul(z2[:], q_cur[:], 2.0 * COMPAT)
                    nc.vector.tensor_sub(z1[:], z1[:], z2[:])
                    nc.vector.tensor_add(z1[:], z1[:], logits_sb[:])
                    nc.vector.tensor_reduce(
                        negm[:], z1[:].rearrange("p (y c) -> p y c", c=C),
                        axis=AX.X, op=ALU.max, negate=True,
                    )
                    for y in range(YL):
                        nc.scalar.activation(
                            ez[:, y * C:(y + 1) * C], z1[:, y * C:(y + 1) * C],
                            AF.Exp, bias=negm[:, y:y + 1],
                        )
                    nc.vector.tensor_reduce(
                        s12[:], ez[:].rearrange("p (y c) -> p y c", c=C),
                        axis=AX.X, op=ALU.add,
                    )
                    nc.vector.reciprocal(r12[:], s12[:])
                    for y in range(YL):
                        nc.vector.tensor_scalar_mul(
                            q_nxt[:, y * C:(y + 1) * C], ez[:, y * C:(y + 1) * C],
                            r12[:, y:y + 1],
                        )
                    q_cur, q_nxt = q_nxt, q_cur

            # ---------------- output ----------------
            outs = sb.tile([96, FREE], FP32)
            nc.vector.tensor_scalar_add(outs[:], q_cur[:], ce_bcast[:])
            nc.sync.dma_start(out_d[:], outs[:])

    nc.compile()
    return nc


def host_prepare(logits, labels, image):
    """Build the 8 per-core input maps."""
    logits_nc = np.ascontiguousarray(
        np.asarray(logits, np.float32)[0].reshape(C, N).T)      # [N, C]
    labels_n = np.asarray(labels).reshape(N).astype(np.int64)
    rgb = np.asarray(image, np.float32)[0].transpose(1, 2, 0).reshape(N, 3)

    onehot = np.zeros((N, C), np.float32)
    onehot[np.arange(N), labels_n] = 1.0

    yy, xx = np.meshgrid(np.arange(H), np.arange(W), indexing="ij")
    pos = np.stack([yy, xx], -1).reshape(N, 2).astype(np.float32)
    cpos = pos - pos.mean(0)
    f5 = np.concatenate([cpos / 3.0, rgb / 255.0], 1)            # [N, 5]
    sq = (f5 * f5).sum(1)

    a = np.arange(H, dtype=np.float32)
    G = np.exp(-0.5 * ((a[:, None] - a[None, :]) / 64.0) ** 2).astype(np.float32)
    ident = np.eye(32, dtype=np.float32)

    def to_strip_dom(arr_nc, r):
        # [N, C] global rows -> [96, (y, c)] strip-domain layout
        s = arr_nc[r * STRIP:(r + 1) * STRIP].reshape(YL, 96, C)
        return np.ascontiguousarray(s.transpose(1, 0, 2).reshape(96, FREE))

    in_maps = []
    for r in range(NCORES):
        j = np.arange(r * STRIP - PAD, r * STRIP - PAD + BAND)
        valid = (j >= 0) & (j < N)
        jc = np.clip(j, 0, N - 1)
        ft = np.zeros((6, BAND), np.float32)
        ft[0:5, valid] = f5[jc[valid]].T
        ft[5, valid] = 1.0
        i_idx = np.arange(r * STRIP, (r + 1) * STRIP)
        rt = np.concatenate(
            [f5[i_idx].T, (-0.5 * sq[i_idx])[None, :]], 0).astype(np.float32)
        biasb = np.where(valid, -0.5 * sq[jc], -1e9).astype(np.float32)
        biasb = np.ascontiguousarray(biasb.reshape(BT, 128).T)   # [128, BT]
        info = np.array([[r * STRIP, r * FREE]], np.int32)
        in_maps.append({
            "logits_dev": to_strip_dom(logits_nc, r),
            "onehot_dev": to_strip_dom(onehot, r),
            "ft_dev": ft,
            "rt_dev": np.ascontiguousarray(rt),
            "biasb_dev": biasb,
            "g_dev": G,
            "ident_dev": ident,
            "info_dev": info,
        })
    return in_maps


def assemble_output(results):
    # per-core [96, FREE] strip-domain -> [1, C, H, W]
    q = np.zeros((N, C), np.float32)
    for r in range(NCORES):
        s = results[r]["out_strip"].reshape(96, YL, C).transpose(1, 0, 2)
        q[r * STRIP:(r + 1) * STRIP] = s.reshape(STRIP, C)
    return np.ascontiguousarray(q.T.reshape(1, C, H, W))


def kernel(logits, labels, image, num_classes, _trace=False):
    global _compiled
    if _compiled is None:
        _compiled = build_nc()
    in_maps = host_prepare(logits, labels, image)
    res = run_bass_kernel_spmd(
        _compiled, in_maps, list(range(NCORES)), trace=_trace)
    out = assemble_output(res.results)
    if _trace:
        return out, res
    return out
